# revision 17
# baseline (speedup 1.0000x reference)
"""AuthorGroupAttention Trainium2 kernel (v2).

Data-parallel over batch: 8 samples -> 8 NeuronCores, one sample per core.
Per-sample routing (reader_token) is resolved on the host by gathering the
routed per-group weights into per-core combined projection weights.

v2 layout (all matmuls bf16; PE cost = moving-free-size rows):
  - Q/K projections per head h produce [128=(d_gen|d_rdr), T] tiles from
    host-combined weights (pumped into the attn loop, per-nh PSUM tiles).
  - scores^T[s,t] via row-packed K=64 matmuls (gen rows 0-63, rdr 64-127),
    exp on ScalarE from PSUM with scale=D**-0.5, output bf16 into
    ex[128, s, path, t] (one strided activation per (th, s)).
  - attention REORIENTED: stationary = ex tile [128 s, 128 t-cols], moving =
    v [128 s, 64 d] -> psum [128 t, 64 d] at 64 rows per (s, t-tile) - half
    the moving rows of the v-stationary orientation.  Z comes from separate
    ones-moving matmuls ([128,1] dest); the ones values 2.0 (gen) and 20.0
    (rdr) pre-fold the path weights (0.45 = 0.9/2, 0.05 = 0.1/2) so
    1/Z' needs only a x0.9 compensation on the gen side.
  - combine on DVE with per-partition scalars (t is now the partition dim):
    comb = (pag * rec_g) * 0.9 + (par * rec_r), written bf16.
  - comb [t, d] is transposed back to [e, t] by PE transpose matmuls into a
    bf16 PSUM tile (pumped into the NEXT head-pair's score loop, sharing the
    score PSUM ring), drained by the Pool engine.
  - output projection streams Wo.T per o-tile with the v-bias folded into
    the output bias on the host (probs rows sum to 0.5).
"""

import os
import sys

for _p in ("/opt/trn_rl_repo",):
    if os.path.isdir(_p) and _p not in sys.path:
        sys.path.insert(0, _p)

import numpy as np
import ml_dtypes

import concourse.bass as bass
import concourse.mybir as mybir
from concourse import bacc
from concourse.tile import TileContext
from concourse.bass_utils import run_bass_kernel_spmd

B, T, E, H, G = 8, 1024, 1024, 16, 4
D = E // H  # 64
SCALING = float(D) ** -0.5
W_G = 0.9 / 2.0  # generic path weight after the /2
W_R = 0.1 / 2.0  # reader path weight after the /2
ONE_G = 2.0   # gen Z ones value; rec*0.9 recovers W_G/Z
ONE_R = 20.0  # rdr Z ones value == 1/W_R exactly in bf16

F32 = mybir.dt.float32
BF16 = mybir.dt.bfloat16
EO = E // 128  # 8 e-chunks
SO = T // 128  # 8 s-tiles
OO = E // 128  # 8 o-tiles
MULT = mybir.AluOpType.mult
ADD = mybir.AluOpType.add


def build_nc():
    nc = bacc.Bacc(name="author_group_attention")

    hsT = nc.dram_tensor("hsT", [E, T], BF16, kind="ExternalInput")
    wq = nc.dram_tensor("wq", [E, H, 128], BF16, kind="ExternalInput")
    wk = nc.dram_tensor("wk", [E, H, 128], BF16, kind="ExternalInput")
    wv = nc.dram_tensor("wv", [E, E], BF16, kind="ExternalInput")
    wo = nc.dram_tensor("wo", [E, E], BF16, kind="ExternalInput")
    bqk = nc.dram_tensor("bqk", [128, 2 * H], F32, kind="ExternalInput")
    bo = nc.dram_tensor("bo", [128, OO], F32, kind="ExternalInput")
    ident = nc.dram_tensor("ident", [128, 128], BF16, kind="ExternalInput")
    outT = nc.dram_tensor("outT", [E, T], F32, kind="ExternalOutput")

    with TileContext(nc) as tc:
        from contextlib import ExitStack

        with ExitStack() as stack:
            const = stack.enter_context(tc.tile_pool(name="const", bufs=1))

            hsT_sb = const.tile([128, EO, T], BF16, tag="hsT")
            hsT_r = hsT.rearrange("(eo ep) t -> ep eo t", ep=128)
            # v in natural [s, h, d] layout: moving operand of the attn mms
            v_sb = const.tile([128, SO, H, D], BF16, tag="v")
            comb_tiles = [
                const.tile([128, T], BF16, tag=f"comb{eo}", name=f"comb{eo}")
                for eo in range(EO)
            ]
            bqk_sb = const.tile([128, 2 * H], F32, tag="bqk")
            bo_sb = const.tile([128, OO], F32, tag="bo")
            ident_sb = const.tile([128, 128], BF16, tag="ident")
            ones_sb = const.tile([128, 2], BF16, tag="ones")
            nc.vector.memset(ones_sb[:, 0:1], ONE_G)
            nc.vector.memset(ones_sb[:, 1:2], ONE_R)

            wpool = stack.enter_context(tc.tile_pool(name="wqk", bufs=3))
            qkpool = stack.enter_context(tc.tile_pool(name="qk", bufs=2))
            ppsum = stack.enter_context(
                tc.tile_pool(name="ppsum", bufs=1, space="PSUM")
            )

            def proj_steps(h, which):
                """Projection of combined Q or K for head h as a list of
                emission steps (pumped into other loops). Per-nh PSUM tiles
                (1 bank) with the drain folded into the last chunk step."""
                wt = wpool.tile([128, EO, 128], BF16, tag="w")
                srcw = wq if which == "q" else wk
                nc.sync.dma_start(
                    wt[:], srcw[:, h, :].rearrange("(eo ep) m -> ep eo m", ep=128)
                )
                dst = qkpool.tile([128, T], BF16, tag=which)
                bias_col = 2 * h if which == "q" else 2 * h + 1
                state = {}

                def mk_mm(nh, eo):
                    def step():
                        if eo == 0:
                            state[0] = ppsum.tile(
                                [128, 512], F32, tag="proj", name=f"pp{which}{nh}"
                            )
                        nc.tensor.matmul(
                            state[0][:],
                            wt[:, eo, :],
                            hsT_sb[:, eo, nh * 512 : (nh + 1) * 512],
                            start=(eo == 0),
                            stop=(eo == EO - 1),
                        )
                        if eo == EO - 1:
                            nc.vector.tensor_scalar_add(
                                dst[:, nh * 512 : (nh + 1) * 512],
                                state[0][:],
                                bqk_sb[:, bias_col : bias_col + 1],
                            )
                    return step

                return dst, [mk_mm(nh, eo) for nh in range(2) for eo in range(EO)]

            # ---------------- v projection (natural layout [s, o]) ---------
            with tc.tile_pool(name="wvp", bufs=1) as wvp, tc.tile_pool(
                name="vpsum", bufs=3, space="PSUM"
            ) as vpsum:
                wv_sb = wvp.tile([128, EO, E], BF16, tag="wv")
                wv_r = wv.rearrange("(eo ep) o -> ep eo o", ep=128)
                for eo in range(EO):
                    nc.sync.dma_start(hsT_sb[:, eo], hsT_r[:, eo])
                    nc.sync.dma_start(wv_sb[:, eo], wv_r[:, eo])
                nc.sync.dma_start(bqk_sb[:], bqk[:])
                nc.sync.dma_start(ident_sb[:], ident[:])
                Qh, steps_q0 = proj_steps(0, "q")
                Kh, steps_k0 = proj_steps(0, "k")
                qk0_pump = steps_q0 + steps_k0
                nc.sync.dma_start(bo_sb[:], bo[:])
                # s-tiles in groups of 3 with eo-inner emission
                for g0 in range(0, SO, 3):
                    grp = list(range(g0, min(g0 + 3, SO)))
                    pvs = {}
                    for so in grp:
                        pvs[so] = vpsum.tile([128, T], F32, tag="vproj",
                                             name=f"pv{so}")
                    for eo in range(EO):
                        for so in grp:
                            for nh in range(2):
                                nc.tensor.matmul(
                                    pvs[so][:, nh * 512 : (nh + 1) * 512],
                                    hsT_sb[:, eo, so * 128 : (so + 1) * 128],
                                    wv_sb[:, eo, nh * 512 : (nh + 1) * 512],
                                    start=(eo == 0),
                                    stop=(eo == EO - 1),
                                )
                        for _ in range(2 if g0 >= 6 else 1):
                            if qk0_pump:
                                qk0_pump.pop(0)()
                    for so in grp:
                        nc.vector.tensor_copy(v_sb[:, so], pvs[so][:])

                while qk0_pump:
                    qk0_pump.pop(0)()

            # ---------------- attention main loop ---------------------------
            with ExitStack() as attn_stack:
                expp = attn_stack.enter_context(tc.tile_pool(name="exp", bufs=2))
                zbp = attn_stack.enter_context(tc.tile_pool(name="zb", bufs=2))
                ctp = attn_stack.enter_context(tc.tile_pool(name="ct", bufs=4))
                spsum = attn_stack.enter_context(
                    tc.tile_pool(name="spsum", bufs=2, space="PSUM")
                )
                apsum = attn_stack.enter_context(
                    tc.tile_pool(name="apsum", bufs=1, space="PSUM")
                )

                comb_ts = {}  # h -> comb_t tile [128 t, 8 j, 64 d]

                def run_transposes(p):
                    """Transpose head-pair p's comb_t [t, d] tiles into a
                    bf16 PSUM tile [128=(d|d), T] (1 bank, shares the "ag"
                    ring slot), then drain it to comb_tiles[p] on DVE."""
                    tp = apsum.tile([128, T], BF16, tag="ag", name=f"tp{p}")
                    for hh in (2 * p, 2 * p + 1):
                        band = 64 * (hh % 2)
                        for j in range(SO):
                            nc.tensor.transpose(
                                tp[band : band + 64, j * 128 : (j + 1) * 128],
                                comb_ts[hh][:, j, :],
                                ident_sb[:],
                                tile_position=(0, band),
                            )
                    nc.vector.tensor_copy(comb_tiles[p][:], tp[:])
                    del comb_ts[2 * p]
                    del comb_ts[2 * p + 1]

                for h in range(H):
                    pump = []
                    if h + 1 < H:
                        nextQ, steps_q = proj_steps(h + 1, "q")
                        nextK, steps_k = proj_steps(h + 1, "k")
                        pump = steps_q + steps_k

                    ex = expp.tile([128, SO, 2, T], BF16, tag="ex")
                    zbuf = zbp.tile([128, 16], F32, tag="zb")
                    comb_t = ctp.tile([128, SO, D], BF16, tag="ct")
                    comb_ts[h] = comb_t

                    # ---- scores + exp ----
                    for th in range(2):
                        if th == 1 and h >= 2 and h % 2 == 0:
                            # both comb_t of pair (h-2)//2 are final by now
                            run_transposes(h // 2 - 1)
                        tsl = slice(th * 512, (th + 1) * 512)
                        for s in range(SO):
                            ps = spsum.tile([128, T], F32, tag="sc")
                            ssl = slice(s * 128, (s + 1) * 128)
                            nc.tensor.matmul(
                                ps[:, 0:512],
                                Kh[0:64, ssl],
                                Qh[0:64, tsl],
                                start=True,
                                stop=True,
                            )
                            nc.tensor.matmul(
                                ps[:, 512:1024],
                                Kh[64:128, ssl],
                                Qh[64:128, tsl],
                                start=True,
                                stop=True,
                            )
                            nc.scalar.activation(
                                ex[:, s, :, th * 512 : (th + 1) * 512],
                                ps[:],
                                mybir.ActivationFunctionType.Exp,
                                scale=SCALING,
                            )
                            for _ in range(2):
                                if pump:
                                    pump.pop(0)()

                    # ---- attention (reoriented) + Z + combine ----
                    pag = apsum.tile([128, 512], F32, tag="ag")
                    par = apsum.tile([128, 512], F32, tag="ar")
                    zt = apsum.tile([128, 16], F32, tag="zt")
                    pdst = {0: pag, 1: par}
                    for j in range(SO):
                        jt = slice(j * 128, (j + 1) * 128)
                        jd = slice(j * 64, (j + 1) * 64)
                        for path in range(2):
                            for s in range(SO):
                                exs = ex[:, s, path, jt]
                                nc.tensor.matmul(
                                    pdst[path][:, jd],
                                    exs,
                                    v_sb[:, s, h, :],
                                    start=(s == 0),
                                    stop=(s == SO - 1),
                                )
                                nc.tensor.matmul(
                                    zt[:, 2 * j + path : 2 * j + path + 1],
                                    exs,
                                    ones_sb[:, path : path + 1],
                                    start=(s == 0),
                                    stop=(s == SO - 1),
                                )
                            if pump:
                                pump.pop(0)()
                        if j == 3 or j == 7:
                            q0 = (j // 4) * 4
                            nc.vector.reciprocal(
                                zbuf[:, 2 * q0 : 2 * q0 + 8],
                                zt[:, 2 * q0 : 2 * q0 + 8],
                            )
                            for jj in range(q0, q0 + 4):
                                jjd = slice(jj * 64, (jj + 1) * 64)
                                nc.vector.tensor_scalar(
                                    comb_t[:, jj, :],
                                    pag[:, jjd],
                                    zbuf[:, 2 * jj : 2 * jj + 1],
                                    0.9,
                                    MULT,
                                    MULT,
                                )
                                nc.vector.scalar_tensor_tensor(
                                    comb_t[:, jj, :],
                                    par[:, jjd],
                                    zbuf[:, 2 * jj + 1 : 2 * jj + 2],
                                    comb_t[:, jj, :],
                                    MULT,
                                    ADD,
                                )
                    while pump:
                        pump.pop(0)()
                    if h + 1 < H:
                        Qh, Kh = nextQ, nextK

                # tail: transpose the last head pair
                run_transposes(H // 2 - 1)

            # ---------------- output projection -----------------------------
            with tc.tile_pool(name="tail", bufs=8) as tailp, tc.tile_pool(
                name="outsb", bufs=2
            ) as outp, tc.tile_pool(name="opsum", bufs=2, space="PSUM") as opsum:
                wo_r = wo.rearrange("(eo ep) (oo m) -> oo ep eo m", ep=128, m=128)
                wts = []
                for jj in range(OO):
                    wt = tailp.tile([128, EO, 128], BF16, tag="wo", name=f"wo{jj}")
                    nc.sync.dma_start(wt[:], wo_r[jj])
                    wts.append(wt)
                for jj in range(OO):
                    wt = wts[jj]
                    po = opsum.tile([128, T], F32, tag="oproj")
                    ot = outp.tile([128, T], F32, tag="ot")
                    for nh in range(2):
                        for eo in range(EO):
                            nc.tensor.matmul(
                                po[:, nh * 512 : (nh + 1) * 512],
                                wt[:, eo, :],
                                comb_tiles[eo][:, nh * 512 : (nh + 1) * 512],
                                start=(eo == 0),
                                stop=(eo == EO - 1),
                            )
                        nc.vector.tensor_scalar_add(
                            ot[:, nh * 512 : (nh + 1) * 512],
                            po[:, nh * 512 : (nh + 1) * 512],
                            bo_sb[:, jj : jj + 1],
                        )
                        nc.sync.dma_start(
                            outT[jj * 128 : (jj + 1) * 128, nh * 512 : (nh + 1) * 512],
                            ot[:, nh * 512 : (nh + 1) * 512],
                        )

    nc.finalize()
    return nc


_NC_CACHE = {}


def get_nc():
    if "nc" not in _NC_CACHE:
        _NC_CACHE["nc"] = build_nc()
    return _NC_CACHE["nc"]


def _host_prep(hidden_states, reader_token, Wq, bq, Wk, bk, Wv, bv, Wo, bo,
               RWq, Rbq, RWk, Rbk, RWv, Rbv):
    """Build the 8 per-core input maps (numpy only)."""
    f = np.float32
    bf = ml_dtypes.bfloat16
    hs = np.asarray(hidden_states, f)
    tok = np.asarray(reader_token).astype(np.int64)
    WqT = np.ascontiguousarray(np.asarray(Wq, f).T)  # [e, o]
    WkT = np.ascontiguousarray(np.asarray(Wk, f).T)
    WvT = np.ascontiguousarray(np.asarray(Wv, f).T).astype(bf)
    WoT = np.ascontiguousarray(np.asarray(Wo, f).T).astype(bf)
    RWqT = np.transpose(np.asarray(RWq, f), (0, 2, 1))  # [g, e, o]
    RWkT = np.transpose(np.asarray(RWk, f), (0, 2, 1))
    bq = np.asarray(bq, f); bk = np.asarray(bk, f)
    bv = np.asarray(bv, f); bo_ = np.asarray(bo, f)
    Rbq = np.asarray(Rbq, f); Rbk = np.asarray(Rbk, f)

    # v-bias folds into the output bias: probs rows sum to 0.5, so attention
    # over (v + bv) adds 0.5*bv to every attn row -> out += 0.5 * bv @ Wo.T
    bo_eff = bo_ + 0.5 * (np.asarray(Wo, f) @ bv)
    bo_t = np.ascontiguousarray(bo_eff.reshape(OO, 128).T)  # [128, oo]

    # shared [e, h, 64] views of the generic weights
    WqT_h = WqT.reshape(E, H, D)
    WkT_h = WkT.reshape(E, H, D)

    ident = np.eye(128, dtype=f).astype(bf)

    in_maps = []
    percore = {}
    for b in range(B):
        g = int(tok[b])
        if g not in percore:
            wqc = np.empty((E, H, 128), f)
            wqc[:, :, :D] = WqT_h
            wqc[:, :, D:] = RWqT[g].reshape(E, H, D)
            wkc = np.empty((E, H, 128), f)
            wkc[:, :, :D] = WkT_h
            wkc[:, :, D:] = RWkT[g].reshape(E, H, D)
            # per-head combined biases: col 2h = [bq_h|Rbq_h], col 2h+1 = k
            bqk_t = np.empty((128, 2 * H), f)
            bqk_t[:D, 0::2] = bq.reshape(H, D).T
            bqk_t[D:, 0::2] = Rbq[g].reshape(H, D).T
            bqk_t[:D, 1::2] = bk.reshape(H, D).T
            bqk_t[D:, 1::2] = Rbk[g].reshape(H, D).T
            percore[g] = (wqc.astype(bf), wkc.astype(bf), bqk_t)
        wqc, wkc, bqk_t = percore[g]
        in_maps.append(
            {
                "hsT": np.ascontiguousarray(hs[b].T).astype(bf),
                "wq": wqc,
                "wk": wkc,
                "wv": WvT,
                "wo": WoT,
                "bqk": bqk_t,
                "bo": bo_t,
                "ident": ident,
            }
        )
    return in_maps


def kernel(**inputs) -> np.ndarray:
    in_maps = _host_prep(**inputs)
    nc = get_nc()
    res = run_bass_kernel_spmd(nc, in_maps, list(range(B)))
    out = np.stack([res.results[c]["outT"].T for c in range(B)], axis=0)
    return np.ascontiguousarray(out.astype(np.float32))


if __name__ == "__main__":
    rng = np.random.default_rng(0)
    ins = {
        "hidden_states": rng.standard_normal((B, T, E), dtype=np.float32),
        "reader_token": rng.integers(0, G, size=(B,)).astype(np.int32),
        "Wq": rng.standard_normal((E, E), dtype=np.float32) * 0.02,
        "bq": np.zeros(E, np.float32),
        "Wk": rng.standard_normal((E, E), dtype=np.float32) * 0.02,
        "bk": np.zeros(E, np.float32),
        "Wv": rng.standard_normal((E, E), dtype=np.float32) * 0.02,
        "bv": np.zeros(E, np.float32),
        "Wo": rng.standard_normal((E, E), dtype=np.float32) * 0.02,
        "bo": np.zeros(E, np.float32),
        "RWq": rng.standard_normal((G, E, E), dtype=np.float32) * 0.02,
        "Rbq": np.zeros((G, E), np.float32),
        "RWk": rng.standard_normal((G, E, E), dtype=np.float32) * 0.02,
        "Rbk": np.zeros((G, E), np.float32),
        "RWv": rng.standard_normal((G, E, E), dtype=np.float32) * 0.02,
        "Rbv": np.zeros((G, E), np.float32),
    }
    out = kernel(**ins)
    print("out", out.shape, out.dtype, float(np.abs(out).max()))


# revision 18
# speedup vs baseline: 1.0055x; 1.0055x over previous
"""AuthorGroupAttention Trainium2 kernel (v2).

Data-parallel over batch: 8 samples -> 8 NeuronCores, one sample per core.
Per-sample routing (reader_token) is resolved on the host by gathering the
routed per-group weights into per-core combined projection weights.

v2 layout (all matmuls bf16; PE cost = moving-free-size rows):
  - Q/K projections per head h produce [128=(d_gen|d_rdr), T] tiles from
    host-combined weights (pumped into the attn loop, per-nh PSUM tiles).
  - scores^T[s,t] via row-packed K=64 matmuls (gen rows 0-63, rdr 64-127),
    exp on ScalarE from PSUM with scale=D**-0.5, output bf16 into
    ex[128, s, path, t] (one strided activation per (th, s)).
  - attention REORIENTED: stationary = ex tile [128 s, 128 t-cols], moving =
    v [128 s, 64 d] -> psum [128 t, 64 d] at 64 rows per (s, t-tile) - half
    the moving rows of the v-stationary orientation.  Z comes from separate
    ones-moving matmuls ([128,1] dest); the ones values 2.0 (gen) and 20.0
    (rdr) pre-fold the path weights (0.45 = 0.9/2, 0.05 = 0.1/2) so
    1/Z' needs only a x0.9 compensation on the gen side.
  - combine on DVE with per-partition scalars (t is now the partition dim):
    comb = (pag * rec_g) * 0.9 + (par * rec_r), written bf16.
  - comb [t, d] is transposed back to [e, t] by PE transpose matmuls into a
    bf16 PSUM tile (pumped into the NEXT head-pair's score loop, sharing the
    score PSUM ring), drained by the Pool engine.
  - output projection streams Wo.T per o-tile with the v-bias folded into
    the output bias on the host (probs rows sum to 0.5).
"""

import os
import sys

for _p in ("/opt/trn_rl_repo",):
    if os.path.isdir(_p) and _p not in sys.path:
        sys.path.insert(0, _p)

import numpy as np
import ml_dtypes

import concourse.bass as bass
import concourse.mybir as mybir
from concourse import bacc
from concourse.tile import TileContext
from concourse.bass_utils import run_bass_kernel_spmd

B, T, E, H, G = 8, 1024, 1024, 16, 4
D = E // H  # 64
SCALING = float(D) ** -0.5
W_G = 0.9 / 2.0  # generic path weight after the /2
W_R = 0.1 / 2.0  # reader path weight after the /2
ONE_G = 2.0   # gen Z ones value; rec*0.9 recovers W_G/Z
ONE_R = 20.0  # rdr Z ones value == 1/W_R exactly in bf16

F32 = mybir.dt.float32
BF16 = mybir.dt.bfloat16
EO = E // 128  # 8 e-chunks
SO = T // 128  # 8 s-tiles
OO = E // 128  # 8 o-tiles
MULT = mybir.AluOpType.mult
ADD = mybir.AluOpType.add


def build_nc():
    nc = bacc.Bacc(name="author_group_attention")

    hsT = nc.dram_tensor("hsT", [E, T], BF16, kind="ExternalInput")
    wq = nc.dram_tensor("wq", [E, H, 128], BF16, kind="ExternalInput")
    wk = nc.dram_tensor("wk", [E, H, 128], BF16, kind="ExternalInput")
    wv = nc.dram_tensor("wv", [E, E], BF16, kind="ExternalInput")
    wo = nc.dram_tensor("wo", [E, E], BF16, kind="ExternalInput")
    bqk = nc.dram_tensor("bqk", [128, 2 * H], F32, kind="ExternalInput")
    bo = nc.dram_tensor("bo", [128, OO], F32, kind="ExternalInput")
    ident = nc.dram_tensor("ident", [128, 128], BF16, kind="ExternalInput")
    outT = nc.dram_tensor("outT", [E, T], F32, kind="ExternalOutput")

    with TileContext(nc) as tc:
        from contextlib import ExitStack

        with ExitStack() as stack:
            const = stack.enter_context(tc.tile_pool(name="const", bufs=1))

            hsT_sb = const.tile([128, EO, T], BF16, tag="hsT")
            hsT_r = hsT.rearrange("(eo ep) t -> ep eo t", ep=128)
            # v in natural [s, h, d] layout: moving operand of the attn mms
            v_sb = const.tile([128, SO, H, D], BF16, tag="v")
            comb_tiles = [
                const.tile([128, T], BF16, tag=f"comb{eo}", name=f"comb{eo}")
                for eo in range(EO)
            ]
            bqk_sb = const.tile([128, 2 * H], F32, tag="bqk")
            bo_sb = const.tile([128, OO], F32, tag="bo")
            ident_sb = const.tile([128, 128], BF16, tag="ident")
            ones_sb = const.tile([128, 2], BF16, tag="ones")
            nc.vector.memset(ones_sb[:, 0:1], ONE_G)
            nc.vector.memset(ones_sb[:, 1:2], ONE_R)

            wpool = stack.enter_context(tc.tile_pool(name="wqk", bufs=3))
            qkpool = stack.enter_context(tc.tile_pool(name="qk", bufs=2))
            ppsum = stack.enter_context(
                tc.tile_pool(name="ppsum", bufs=1, space="PSUM")
            )

            def proj_steps(h, which):
                """Projection of combined Q or K for head h as a list of
                emission steps (pumped into other loops). Per-nh PSUM tiles
                (1 bank) with the drain folded into the last chunk step."""
                wt = wpool.tile([128, EO, 128], BF16, tag="w")
                srcw = wq if which == "q" else wk
                nc.sync.dma_start(
                    wt[:], srcw[:, h, :].rearrange("(eo ep) m -> ep eo m", ep=128)
                )
                dst = qkpool.tile([128, T], BF16, tag=which)
                bias_col = 2 * h if which == "q" else 2 * h + 1
                state = {}

                def mk_mm(nh, eo):
                    def step():
                        if eo == 0:
                            state[0] = ppsum.tile(
                                [128, 512], F32, tag="proj", name=f"pp{which}{nh}"
                            )
                        nc.tensor.matmul(
                            state[0][:],
                            wt[:, eo, :],
                            hsT_sb[:, eo, nh * 512 : (nh + 1) * 512],
                            start=(eo == 0),
                            stop=(eo == EO - 1),
                        )
                        if eo == EO - 1:
                            nc.vector.tensor_scalar_add(
                                dst[:, nh * 512 : (nh + 1) * 512],
                                state[0][:],
                                bqk_sb[:, bias_col : bias_col + 1],
                            )
                    return step

                return dst, [mk_mm(nh, eo) for nh in range(2) for eo in range(EO)]

            # ---------------- v projection (natural layout [s, o]) ---------
            with tc.tile_pool(name="wvp", bufs=1) as wvp, tc.tile_pool(
                name="vpsum", bufs=3, space="PSUM"
            ) as vpsum:
                wv_sb = wvp.tile([128, EO, E], BF16, tag="wv")
                wv_r = wv.rearrange("(eo ep) o -> ep eo o", ep=128)
                for eo in range(EO):
                    nc.sync.dma_start(hsT_sb[:, eo], hsT_r[:, eo])
                    nc.sync.dma_start(wv_sb[:, eo], wv_r[:, eo])
                nc.sync.dma_start(bqk_sb[:], bqk[:])
                nc.sync.dma_start(ident_sb[:], ident[:])
                Qh, steps_q0 = proj_steps(0, "q")
                Kh, steps_k0 = proj_steps(0, "k")
                qk0_pump = steps_q0 + steps_k0
                nc.sync.dma_start(bo_sb[:], bo[:])
                # s-tiles in groups of 3 with eo-inner emission
                for g0 in range(0, SO, 3):
                    grp = list(range(g0, min(g0 + 3, SO)))
                    pvs = {}
                    for so in grp:
                        pvs[so] = vpsum.tile([128, T], F32, tag="vproj",
                                             name=f"pv{so}")
                    for eo in range(EO):
                        for so in grp:
                            for nh in range(2):
                                nc.tensor.matmul(
                                    pvs[so][:, nh * 512 : (nh + 1) * 512],
                                    hsT_sb[:, eo, so * 128 : (so + 1) * 128],
                                    wv_sb[:, eo, nh * 512 : (nh + 1) * 512],
                                    start=(eo == 0),
                                    stop=(eo == EO - 1),
                                )
                        for _ in range(2 if g0 >= 6 else 1):
                            if qk0_pump:
                                qk0_pump.pop(0)()
                    for so in grp:
                        nc.vector.tensor_copy(v_sb[:, so], pvs[so][:])

                while qk0_pump:
                    qk0_pump.pop(0)()

            # ---------------- attention main loop ---------------------------
            with ExitStack() as attn_stack:
                expp = attn_stack.enter_context(tc.tile_pool(name="exp", bufs=2))
                zbp = attn_stack.enter_context(tc.tile_pool(name="zb", bufs=2))
                ctp = attn_stack.enter_context(tc.tile_pool(name="ct", bufs=4))
                spsum = attn_stack.enter_context(
                    tc.tile_pool(name="spsum", bufs=2, space="PSUM")
                )
                apsum = attn_stack.enter_context(
                    tc.tile_pool(name="apsum", bufs=1, space="PSUM")
                )

                comb_ts = {}  # h -> comb_t tile [128 t, 8 j, 64 d]

                def run_transposes(p):
                    """Transpose head-pair p's comb_t [t, d] tiles into a
                    bf16 PSUM tile [128=(d|d), T] (1 bank, shares the "ag"
                    ring slot), then drain it to comb_tiles[p] on DVE."""
                    tp = apsum.tile([128, T], BF16, tag="ag", name=f"tp{p}")
                    for hh in (2 * p, 2 * p + 1):
                        band = 64 * (hh % 2)
                        for j in range(SO):
                            nc.tensor.transpose(
                                tp[band : band + 64, j * 128 : (j + 1) * 128],
                                comb_ts[hh][:, j, :],
                                ident_sb[:],
                                tile_position=(0, band),
                            )
                    nc.vector.tensor_copy(comb_tiles[p][:], tp[:])
                    del comb_ts[2 * p]
                    del comb_ts[2 * p + 1]

                for h in range(H):
                    pump = []
                    if h + 1 < H:
                        nextQ, steps_q = proj_steps(h + 1, "q")
                        nextK, steps_k = proj_steps(h + 1, "k")
                        pump = steps_q + steps_k

                    ex = expp.tile([128, SO, 2, T], BF16, tag="ex")
                    zbuf = zbp.tile([128, 16], F32, tag="zb")
                    comb_t = ctp.tile([128, SO, D], BF16, tag="ct")
                    comb_ts[h] = comb_t

                    # ---- scores + exp ----
                    for th in range(2):
                        if th == 1 and h >= 2 and h % 2 == 0:
                            # both comb_t of pair (h-2)//2 are final by now
                            run_transposes(h // 2 - 1)
                        tsl = slice(th * 512, (th + 1) * 512)
                        for s in range(SO):
                            ps = spsum.tile([128, T], F32, tag="sc")
                            ssl = slice(s * 128, (s + 1) * 128)
                            nc.tensor.matmul(
                                ps[:, 0:512],
                                Kh[0:64, ssl],
                                Qh[0:64, tsl],
                                start=True,
                                stop=True,
                            )
                            nc.tensor.matmul(
                                ps[:, 512:1024],
                                Kh[64:128, ssl],
                                Qh[64:128, tsl],
                                start=True,
                                stop=True,
                            )
                            nc.scalar.activation(
                                ex[:, s, :, th * 512 : (th + 1) * 512],
                                ps[:],
                                mybir.ActivationFunctionType.Exp,
                                scale=SCALING,
                            )
                            for _ in range(2):
                                if pump:
                                    pump.pop(0)()

                    # ---- attention (reoriented) + Z + combine ----
                    pag = apsum.tile([128, 512], F32, tag="ag")
                    par = apsum.tile([128, 512], F32, tag="ar")
                    zt = apsum.tile([128, 16], F32, tag="zt")
                    pdst = {0: pag, 1: par}
                    for j in range(SO):
                        jt = slice(j * 128, (j + 1) * 128)
                        jd = slice(j * 64, (j + 1) * 64)
                        for path in range(2):
                            for s in range(SO):
                                exs = ex[:, s, path, jt]
                                nc.tensor.matmul(
                                    pdst[path][:, jd],
                                    exs,
                                    v_sb[:, s, h, :],
                                    start=(s == 0),
                                    stop=(s == SO - 1),
                                )
                                nc.tensor.matmul(
                                    zt[:, 2 * j + path : 2 * j + path + 1],
                                    exs,
                                    ones_sb[:, path : path + 1],
                                    start=(s == 0),
                                    stop=(s == SO - 1),
                                )
                            if pump:
                                pump.pop(0)()
                        if j == 3 or j == 7:
                            q0 = (j // 4) * 4
                            nc.vector.reciprocal(
                                zbuf[:, 2 * q0 : 2 * q0 + 8],
                                zt[:, 2 * q0 : 2 * q0 + 8],
                            )
                            for jj in range(q0, q0 + 4):
                                jjd = slice(jj * 64, (jj + 1) * 64)
                                nc.vector.tensor_scalar(
                                    comb_t[:, jj, :],
                                    pag[:, jjd],
                                    zbuf[:, 2 * jj : 2 * jj + 1],
                                    0.9,
                                    MULT,
                                    MULT,
                                )
                                nc.vector.scalar_tensor_tensor(
                                    comb_t[:, jj, :],
                                    par[:, jjd],
                                    zbuf[:, 2 * jj + 1 : 2 * jj + 2],
                                    comb_t[:, jj, :],
                                    MULT,
                                    ADD,
                                )
                    while pump:
                        pump.pop(0)()
                    if h + 1 < H:
                        Qh, Kh = nextQ, nextK

                # tail: transpose the last head pair
                run_transposes(H // 2 - 1)

            # ---------------- output projection -----------------------------
            with tc.tile_pool(name="tail", bufs=3) as tailp, tc.tile_pool(
                name="outsb", bufs=2
            ) as outp, tc.tile_pool(name="opsum", bufs=2, space="PSUM") as opsum:
                wo_r = wo.rearrange("(eo ep) (oo m) -> oo ep eo m", ep=128, m=128)
                for jj in range(OO):
                    wt = tailp.tile([128, EO, 128], BF16, tag="wo")
                    nc.sync.dma_start(wt[:], wo_r[jj])
                    po = opsum.tile([128, T], F32, tag="oproj")
                    ot = outp.tile([128, T], F32, tag="ot")
                    for nh in range(2):
                        for eo in range(EO):
                            nc.tensor.matmul(
                                po[:, nh * 512 : (nh + 1) * 512],
                                wt[:, eo, :],
                                comb_tiles[eo][:, nh * 512 : (nh + 1) * 512],
                                start=(eo == 0),
                                stop=(eo == EO - 1),
                            )
                        nc.vector.tensor_scalar_add(
                            ot[:, nh * 512 : (nh + 1) * 512],
                            po[:, nh * 512 : (nh + 1) * 512],
                            bo_sb[:, jj : jj + 1],
                        )
                        nc.sync.dma_start(
                            outT[jj * 128 : (jj + 1) * 128, nh * 512 : (nh + 1) * 512],
                            ot[:, nh * 512 : (nh + 1) * 512],
                        )

    nc.finalize()
    return nc


_NC_CACHE = {}


def get_nc():
    if "nc" not in _NC_CACHE:
        _NC_CACHE["nc"] = build_nc()
    return _NC_CACHE["nc"]


def _host_prep(hidden_states, reader_token, Wq, bq, Wk, bk, Wv, bv, Wo, bo,
               RWq, Rbq, RWk, Rbk, RWv, Rbv):
    """Build the 8 per-core input maps (numpy only)."""
    f = np.float32
    bf = ml_dtypes.bfloat16
    hs = np.asarray(hidden_states, f)
    tok = np.asarray(reader_token).astype(np.int64)
    WqT = np.ascontiguousarray(np.asarray(Wq, f).T)  # [e, o]
    WkT = np.ascontiguousarray(np.asarray(Wk, f).T)
    WvT = np.ascontiguousarray(np.asarray(Wv, f).T).astype(bf)
    WoT = np.ascontiguousarray(np.asarray(Wo, f).T).astype(bf)
    RWqT = np.transpose(np.asarray(RWq, f), (0, 2, 1))  # [g, e, o]
    RWkT = np.transpose(np.asarray(RWk, f), (0, 2, 1))
    bq = np.asarray(bq, f); bk = np.asarray(bk, f)
    bv = np.asarray(bv, f); bo_ = np.asarray(bo, f)
    Rbq = np.asarray(Rbq, f); Rbk = np.asarray(Rbk, f)

    # v-bias folds into the output bias: probs rows sum to 0.5, so attention
    # over (v + bv) adds 0.5*bv to every attn row -> out += 0.5 * bv @ Wo.T
    bo_eff = bo_ + 0.5 * (np.asarray(Wo, f) @ bv)
    bo_t = np.ascontiguousarray(bo_eff.reshape(OO, 128).T)  # [128, oo]

    # shared [e, h, 64] views of the generic weights
    WqT_h = WqT.reshape(E, H, D)
    WkT_h = WkT.reshape(E, H, D)

    ident = np.eye(128, dtype=f).astype(bf)

    in_maps = []
    percore = {}
    for b in range(B):
        g = int(tok[b])
        if g not in percore:
            wqc = np.empty((E, H, 128), f)
            wqc[:, :, :D] = WqT_h
            wqc[:, :, D:] = RWqT[g].reshape(E, H, D)
            wkc = np.empty((E, H, 128), f)
            wkc[:, :, :D] = WkT_h
            wkc[:, :, D:] = RWkT[g].reshape(E, H, D)
            # per-head combined biases: col 2h = [bq_h|Rbq_h], col 2h+1 = k
            bqk_t = np.empty((128, 2 * H), f)
            bqk_t[:D, 0::2] = bq.reshape(H, D).T
            bqk_t[D:, 0::2] = Rbq[g].reshape(H, D).T
            bqk_t[:D, 1::2] = bk.reshape(H, D).T
            bqk_t[D:, 1::2] = Rbk[g].reshape(H, D).T
            percore[g] = (wqc.astype(bf), wkc.astype(bf), bqk_t)
        wqc, wkc, bqk_t = percore[g]
        in_maps.append(
            {
                "hsT": np.ascontiguousarray(hs[b].T).astype(bf),
                "wq": wqc,
                "wk": wkc,
                "wv": WvT,
                "wo": WoT,
                "bqk": bqk_t,
                "bo": bo_t,
                "ident": ident,
            }
        )
    return in_maps


def kernel(**inputs) -> np.ndarray:
    in_maps = _host_prep(**inputs)
    nc = get_nc()
    res = run_bass_kernel_spmd(nc, in_maps, list(range(B)))
    out = np.stack([res.results[c]["outT"].T for c in range(B)], axis=0)
    return np.ascontiguousarray(out.astype(np.float32))


if __name__ == "__main__":
    rng = np.random.default_rng(0)
    ins = {
        "hidden_states": rng.standard_normal((B, T, E), dtype=np.float32),
        "reader_token": rng.integers(0, G, size=(B,)).astype(np.int32),
        "Wq": rng.standard_normal((E, E), dtype=np.float32) * 0.02,
        "bq": np.zeros(E, np.float32),
        "Wk": rng.standard_normal((E, E), dtype=np.float32) * 0.02,
        "bk": np.zeros(E, np.float32),
        "Wv": rng.standard_normal((E, E), dtype=np.float32) * 0.02,
        "bv": np.zeros(E, np.float32),
        "Wo": rng.standard_normal((E, E), dtype=np.float32) * 0.02,
        "bo": np.zeros(E, np.float32),
        "RWq": rng.standard_normal((G, E, E), dtype=np.float32) * 0.02,
        "Rbq": np.zeros((G, E), np.float32),
        "RWk": rng.standard_normal((G, E, E), dtype=np.float32) * 0.02,
        "Rbk": np.zeros((G, E), np.float32),
        "RWv": rng.standard_normal((G, E, E), dtype=np.float32) * 0.02,
        "Rbv": np.zeros((G, E), np.float32),
    }
    out = kernel(**ins)
    print("out", out.shape, out.dtype, float(np.abs(out).max()))


# revision 20
# speedup vs baseline: 1.0119x; 1.0064x over previous
"""AuthorGroupAttention Trainium2 kernel (v2).

Data-parallel over batch: 8 samples -> 8 NeuronCores, one sample per core.
Per-sample routing (reader_token) is resolved on the host by gathering the
routed per-group weights into per-core combined projection weights.

v2 layout (all matmuls bf16; PE cost = moving-free-size rows):
  - Q/K projections per head h produce [128=(d_gen|d_rdr), T] tiles from
    host-combined weights (pumped into the attn loop, per-nh PSUM tiles).
  - scores^T[s,t] via row-packed K=64 matmuls (gen rows 0-63, rdr 64-127),
    exp on ScalarE from PSUM with scale=D**-0.5, output bf16 into
    ex[128, s, path, t] (one strided activation per (th, s)).
  - attention REORIENTED: stationary = ex tile [128 s, 128 t-cols], moving =
    v [128 s, 64 d] -> psum [128 t, 64 d] at 64 rows per (s, t-tile) - half
    the moving rows of the v-stationary orientation.  Z comes from separate
    ones-moving matmuls ([128,1] dest); the ones values 2.0 (gen) and 20.0
    (rdr) pre-fold the path weights (0.45 = 0.9/2, 0.05 = 0.1/2) so
    1/Z' needs only a x0.9 compensation on the gen side.
  - combine on DVE with per-partition scalars (t is now the partition dim):
    comb = (pag * rec_g) * 0.9 + (par * rec_r), written bf16.
  - comb [t, d] is transposed back to [e, t] by PE transpose matmuls into
    a bf16 PSUM tile (run mid-scores of the NEXT even head, sharing the gen
    attn PSUM bank slot), drained to SBUF by DVE.  NOTE: only one open PSUM
    accumulation group per 2KB bank is allowed - interleaving j-tile groups
    within a bank silently corrupts results.
  - output projection streams Wo.T per o-tile with the v-bias folded into
    the output bias on the host (probs rows sum to 0.5).
"""

import os
import sys

for _p in ("/opt/trn_rl_repo",):
    if os.path.isdir(_p) and _p not in sys.path:
        sys.path.insert(0, _p)

import numpy as np
import ml_dtypes

import concourse.bass as bass
import concourse.mybir as mybir
from concourse import bacc
from concourse.tile import TileContext
from concourse.bass_utils import run_bass_kernel_spmd

B, T, E, H, G = 8, 1024, 1024, 16, 4
D = E // H  # 64
SCALING = float(D) ** -0.5
W_G = 0.9 / 2.0  # generic path weight after the /2
W_R = 0.1 / 2.0  # reader path weight after the /2
ONE_G = 2.0   # gen Z ones value; rec*0.9 recovers W_G/Z
ONE_R = 20.0  # rdr Z ones value == 1/W_R exactly in bf16

F32 = mybir.dt.float32
BF16 = mybir.dt.bfloat16
EO = E // 128  # 8 e-chunks
SO = T // 128  # 8 s-tiles
OO = E // 128  # 8 o-tiles
MULT = mybir.AluOpType.mult
ADD = mybir.AluOpType.add


def build_nc():
    nc = bacc.Bacc(name="author_group_attention")

    hsT = nc.dram_tensor("hsT", [E, T], BF16, kind="ExternalInput")
    wq = nc.dram_tensor("wq", [E, H, 128], BF16, kind="ExternalInput")
    wk = nc.dram_tensor("wk", [E, H, 128], BF16, kind="ExternalInput")
    wv = nc.dram_tensor("wv", [E, E], BF16, kind="ExternalInput")
    wo = nc.dram_tensor("wo", [E, E], BF16, kind="ExternalInput")
    bqk = nc.dram_tensor("bqk", [128, 2 * H], F32, kind="ExternalInput")
    bo = nc.dram_tensor("bo", [128, OO], F32, kind="ExternalInput")
    ident = nc.dram_tensor("ident", [128, 128], BF16, kind="ExternalInput")
    outT = nc.dram_tensor("outT", [E, T], F32, kind="ExternalOutput")

    with TileContext(nc) as tc:
        from contextlib import ExitStack

        with ExitStack() as stack:
            const = stack.enter_context(tc.tile_pool(name="const", bufs=1))

            hsT_sb = const.tile([128, EO, T], BF16, tag="hsT")
            hsT_r = hsT.rearrange("(eo ep) t -> ep eo t", ep=128)
            # v in natural [s, h, d] layout: moving operand of the attn mms
            v_sb = const.tile([128, SO, H, D], BF16, tag="v")
            comb_tiles = [
                const.tile([128, T], BF16, tag=f"comb{eo}", name=f"comb{eo}")
                for eo in range(EO)
            ]
            bqk_sb = const.tile([128, 2 * H], F32, tag="bqk")
            bo_sb = const.tile([128, OO], F32, tag="bo")
            ident_sb = const.tile([128, 128], BF16, tag="ident")
            ones_sb = const.tile([128, 2], BF16, tag="ones")
            nc.vector.memset(ones_sb[:, 0:1], ONE_G)
            nc.vector.memset(ones_sb[:, 1:2], ONE_R)

            wpool = stack.enter_context(tc.tile_pool(name="wqk", bufs=3))
            qkpool = stack.enter_context(tc.tile_pool(name="qk", bufs=2))
            ppsum = stack.enter_context(
                tc.tile_pool(name="ppsum", bufs=1, space="PSUM")
            )

            def proj_steps(h, which):
                """Projection of combined Q or K for head h as a list of
                emission steps (pumped into other loops). Per-nh PSUM tiles
                (1 bank) with the drain folded into the last chunk step."""
                wt = wpool.tile([128, EO, 128], BF16, tag="w")
                srcw = wq if which == "q" else wk
                nc.sync.dma_start(
                    wt[:], srcw[:, h, :].rearrange("(eo ep) m -> ep eo m", ep=128)
                )
                dst = qkpool.tile([128, T], BF16, tag=which)
                bias_col = 2 * h if which == "q" else 2 * h + 1
                state = {}

                def mk_mm(nh, eo):
                    def step():
                        if eo == 0:
                            state[0] = ppsum.tile(
                                [128, 512], F32, tag="proj", name=f"pp{which}{nh}"
                            )
                        nc.tensor.matmul(
                            state[0][:],
                            wt[:, eo, :],
                            hsT_sb[:, eo, nh * 512 : (nh + 1) * 512],
                            start=(eo == 0),
                            stop=(eo == EO - 1),
                        )
                        if eo == EO - 1:
                            nc.vector.tensor_scalar_add(
                                dst[:, nh * 512 : (nh + 1) * 512],
                                state[0][:],
                                bqk_sb[:, bias_col : bias_col + 1],
                            )
                    return step

                return dst, [mk_mm(nh, eo) for nh in range(2) for eo in range(EO)]

            # ---------------- v projection (natural layout [s, o]) ---------
            with tc.tile_pool(name="wvp", bufs=1) as wvp, tc.tile_pool(
                name="vpsum", bufs=3, space="PSUM"
            ) as vpsum:
                wv_sb = wvp.tile([128, EO, E], BF16, tag="wv")
                wv_r = wv.rearrange("(eo ep) o -> ep eo o", ep=128)
                for eo in range(EO):
                    nc.sync.dma_start(hsT_sb[:, eo], hsT_r[:, eo])
                    nc.sync.dma_start(wv_sb[:, eo], wv_r[:, eo])
                nc.sync.dma_start(bqk_sb[:], bqk[:])
                nc.sync.dma_start(ident_sb[:], ident[:])
                Qh, steps_q0 = proj_steps(0, "q")
                Kh, steps_k0 = proj_steps(0, "k")
                qk0_pump = steps_q0 + steps_k0
                nc.sync.dma_start(bo_sb[:], bo[:])
                # s-tiles in groups of 3 with eo-inner emission
                for g0 in range(0, SO, 3):
                    grp = list(range(g0, min(g0 + 3, SO)))
                    pvs = {}
                    for so in grp:
                        pvs[so] = vpsum.tile([128, T], F32, tag="vproj",
                                             name=f"pv{so}")
                    for eo in range(EO):
                        for so in grp:
                            for nh in range(2):
                                nc.tensor.matmul(
                                    pvs[so][:, nh * 512 : (nh + 1) * 512],
                                    hsT_sb[:, eo, so * 128 : (so + 1) * 128],
                                    wv_sb[:, eo, nh * 512 : (nh + 1) * 512],
                                    start=(eo == 0),
                                    stop=(eo == EO - 1),
                                )
                        for _ in range(2 if g0 >= 3 else 1):
                            if qk0_pump:
                                qk0_pump.pop(0)()
                    for so in grp:
                        nc.scalar.copy(v_sb[:, so], pvs[so][:])

                while qk0_pump:
                    qk0_pump.pop(0)()

            # ---------------- attention main loop ---------------------------
            with ExitStack() as attn_stack:
                expp = attn_stack.enter_context(tc.tile_pool(name="exp", bufs=2))
                zbp = attn_stack.enter_context(tc.tile_pool(name="zb", bufs=2))
                ctp = attn_stack.enter_context(tc.tile_pool(name="ct", bufs=4))
                spsum = attn_stack.enter_context(
                    tc.tile_pool(name="spsum", bufs=2, space="PSUM")
                )
                apsum = attn_stack.enter_context(
                    tc.tile_pool(name="apsum", bufs=1, space="PSUM")
                )

                comb_ts = {}  # h -> comb_t tile [128 t, 8 j, 64 d]

                def run_transposes(p):
                    """Transpose head-pair p's comb_t [t, d] tiles into a
                    bf16 PSUM tile [128=(d|d), T] (1 bank, shares the "ag"
                    ring slot), then drain it to comb_tiles[p] on DVE."""
                    tp = apsum.tile([128, T], BF16, tag="ag", name=f"tp{p}")
                    for hh in (2 * p, 2 * p + 1):
                        band = 64 * (hh % 2)
                        for j in range(SO):
                            nc.tensor.transpose(
                                tp[band : band + 64, j * 128 : (j + 1) * 128],
                                comb_ts[hh][:, j, :],
                                ident_sb[:],
                                tile_position=(0, band),
                            )
                    nc.vector.tensor_copy(comb_tiles[p][:], tp[:])
                    del comb_ts[2 * p]
                    del comb_ts[2 * p + 1]

                for h in range(H):
                    pump = []
                    if h + 1 < H:
                        nextQ, steps_q = proj_steps(h + 1, "q")
                        nextK, steps_k = proj_steps(h + 1, "k")
                        pump = steps_q + steps_k

                    ex = expp.tile([128, SO, 2, T], BF16, tag="ex")
                    zbuf = zbp.tile([128, 16], F32, tag="zb")
                    comb_t = ctp.tile([128, SO, D], BF16, tag="ct")
                    comb_ts[h] = comb_t

                    # ---- scores + exp ----
                    for th in range(2):
                        if th == 1 and h >= 2 and h % 2 == 0:
                            # both comb_t of pair (h-2)//2 are final by now
                            run_transposes(h // 2 - 1)
                        tsl = slice(th * 512, (th + 1) * 512)
                        for s in range(SO):
                            ps = spsum.tile([128, T], F32, tag="sc")
                            ssl = slice(s * 128, (s + 1) * 128)
                            nc.tensor.matmul(
                                ps[:, 0:512],
                                Kh[0:64, ssl],
                                Qh[0:64, tsl],
                                start=True,
                                stop=True,
                            )
                            nc.tensor.matmul(
                                ps[:, 512:1024],
                                Kh[64:128, ssl],
                                Qh[64:128, tsl],
                                start=True,
                                stop=True,
                            )
                            nc.scalar.activation(
                                ex[:, s, :, th * 512 : (th + 1) * 512],
                                ps[:],
                                mybir.ActivationFunctionType.Exp,
                                scale=SCALING,
                            )
                            for _ in range(2):
                                if pump:
                                    pump.pop(0)()

                    # ---- attention (reoriented) + Z + combine ----
                    pag = apsum.tile([128, 512], F32, tag="ag")
                    par = apsum.tile([128, 512], F32, tag="ar")
                    zt = apsum.tile([128, 16], F32, tag="zt")
                    pdst = {0: pag, 1: par}
                    for j in range(SO):
                        jt = slice(j * 128, (j + 1) * 128)
                        jd = slice(j * 64, (j + 1) * 64)
                        for path in range(2):
                            for s in range(SO):
                                exs = ex[:, s, path, jt]
                                nc.tensor.matmul(
                                    pdst[path][:, jd],
                                    exs,
                                    v_sb[:, s, h, :],
                                    start=(s == 0),
                                    stop=(s == SO - 1),
                                )
                                nc.tensor.matmul(
                                    zt[:, 2 * j + path : 2 * j + path + 1],
                                    exs,
                                    ones_sb[:, path : path + 1],
                                    start=(s == 0),
                                    stop=(s == SO - 1),
                                )
                            if pump:
                                pump.pop(0)()
                        if j == 3 or j == 7:
                            q0 = (j // 4) * 4
                            nc.vector.reciprocal(
                                zbuf[:, 2 * q0 : 2 * q0 + 8],
                                zt[:, 2 * q0 : 2 * q0 + 8],
                            )
                            for jj in range(q0, q0 + 4):
                                jjd = slice(jj * 64, (jj + 1) * 64)
                                nc.vector.tensor_scalar(
                                    comb_t[:, jj, :],
                                    pag[:, jjd],
                                    zbuf[:, 2 * jj : 2 * jj + 1],
                                    0.9,
                                    MULT,
                                    MULT,
                                )
                                nc.vector.scalar_tensor_tensor(
                                    comb_t[:, jj, :],
                                    par[:, jjd],
                                    zbuf[:, 2 * jj + 1 : 2 * jj + 2],
                                    comb_t[:, jj, :],
                                    MULT,
                                    ADD,
                                )
                    while pump:
                        pump.pop(0)()
                    if h + 1 < H:
                        Qh, Kh = nextQ, nextK

                # tail: transpose the last head pair
                run_transposes(H // 2 - 1)

            # ---------------- output projection -----------------------------
            with tc.tile_pool(name="tail", bufs=3) as tailp, tc.tile_pool(
                name="outsb", bufs=2
            ) as outp, tc.tile_pool(name="opsum", bufs=2, space="PSUM") as opsum:
                wo_r = wo.rearrange("(eo ep) (oo m) -> oo ep eo m", ep=128, m=128)
                for jj in range(OO):
                    wt = tailp.tile([128, EO, 128], BF16, tag="wo")
                    nc.sync.dma_start(wt[:], wo_r[jj])
                    po = opsum.tile([128, T], F32, tag="oproj")
                    ot = outp.tile([128, T], F32, tag="ot")
                    for nh in range(2):
                        for eo in range(EO):
                            nc.tensor.matmul(
                                po[:, nh * 512 : (nh + 1) * 512],
                                wt[:, eo, :],
                                comb_tiles[eo][:, nh * 512 : (nh + 1) * 512],
                                start=(eo == 0),
                                stop=(eo == EO - 1),
                            )
                        nc.vector.tensor_scalar_add(
                            ot[:, nh * 512 : (nh + 1) * 512],
                            po[:, nh * 512 : (nh + 1) * 512],
                            bo_sb[:, jj : jj + 1],
                        )
                        nc.sync.dma_start(
                            outT[jj * 128 : (jj + 1) * 128, nh * 512 : (nh + 1) * 512],
                            ot[:, nh * 512 : (nh + 1) * 512],
                        )

    nc.finalize()
    return nc


_NC_CACHE = {}


def get_nc():
    if "nc" not in _NC_CACHE:
        _NC_CACHE["nc"] = build_nc()
    return _NC_CACHE["nc"]


def _host_prep(hidden_states, reader_token, Wq, bq, Wk, bk, Wv, bv, Wo, bo,
               RWq, Rbq, RWk, Rbk, RWv, Rbv):
    """Build the 8 per-core input maps (numpy only)."""
    f = np.float32
    bf = ml_dtypes.bfloat16
    hs = np.asarray(hidden_states, f)
    tok = np.asarray(reader_token).astype(np.int64)
    WqT = np.ascontiguousarray(np.asarray(Wq, f).T)  # [e, o]
    WkT = np.ascontiguousarray(np.asarray(Wk, f).T)
    WvT = np.ascontiguousarray(np.asarray(Wv, f).T).astype(bf)
    WoT = np.ascontiguousarray(np.asarray(Wo, f).T).astype(bf)
    RWqT = np.transpose(np.asarray(RWq, f), (0, 2, 1))  # [g, e, o]
    RWkT = np.transpose(np.asarray(RWk, f), (0, 2, 1))
    bq = np.asarray(bq, f); bk = np.asarray(bk, f)
    bv = np.asarray(bv, f); bo_ = np.asarray(bo, f)
    Rbq = np.asarray(Rbq, f); Rbk = np.asarray(Rbk, f)

    # v-bias folds into the output bias: probs rows sum to 0.5, so attention
    # over (v + bv) adds 0.5*bv to every attn row -> out += 0.5 * bv @ Wo.T
    bo_eff = bo_ + 0.5 * (np.asarray(Wo, f) @ bv)
    bo_t = np.ascontiguousarray(bo_eff.reshape(OO, 128).T)  # [128, oo]

    # shared [e, h, 64] views of the generic weights
    WqT_h = WqT.reshape(E, H, D)
    WkT_h = WkT.reshape(E, H, D)

    ident = np.eye(128, dtype=f).astype(bf)

    in_maps = []
    percore = {}
    for b in range(B):
        g = int(tok[b])
        if g not in percore:
            wqc = np.empty((E, H, 128), f)
            wqc[:, :, :D] = WqT_h
            wqc[:, :, D:] = RWqT[g].reshape(E, H, D)
            wkc = np.empty((E, H, 128), f)
            wkc[:, :, :D] = WkT_h
            wkc[:, :, D:] = RWkT[g].reshape(E, H, D)
            # per-head combined biases: col 2h = [bq_h|Rbq_h], col 2h+1 = k
            bqk_t = np.empty((128, 2 * H), f)
            bqk_t[:D, 0::2] = bq.reshape(H, D).T
            bqk_t[D:, 0::2] = Rbq[g].reshape(H, D).T
            bqk_t[:D, 1::2] = bk.reshape(H, D).T
            bqk_t[D:, 1::2] = Rbk[g].reshape(H, D).T
            percore[g] = (wqc.astype(bf), wkc.astype(bf), bqk_t)
        wqc, wkc, bqk_t = percore[g]
        in_maps.append(
            {
                "hsT": np.ascontiguousarray(hs[b].T).astype(bf),
                "wq": wqc,
                "wk": wkc,
                "wv": WvT,
                "wo": WoT,
                "bqk": bqk_t,
                "bo": bo_t,
                "ident": ident,
            }
        )
    return in_maps


def kernel(**inputs) -> np.ndarray:
    in_maps = _host_prep(**inputs)
    nc = get_nc()
    res = run_bass_kernel_spmd(nc, in_maps, list(range(B)))
    out = np.stack([res.results[c]["outT"].T for c in range(B)], axis=0)
    return np.ascontiguousarray(out.astype(np.float32))


if __name__ == "__main__":
    rng = np.random.default_rng(0)
    ins = {
        "hidden_states": rng.standard_normal((B, T, E), dtype=np.float32),
        "reader_token": rng.integers(0, G, size=(B,)).astype(np.int32),
        "Wq": rng.standard_normal((E, E), dtype=np.float32) * 0.02,
        "bq": np.zeros(E, np.float32),
        "Wk": rng.standard_normal((E, E), dtype=np.float32) * 0.02,
        "bk": np.zeros(E, np.float32),
        "Wv": rng.standard_normal((E, E), dtype=np.float32) * 0.02,
        "bv": np.zeros(E, np.float32),
        "Wo": rng.standard_normal((E, E), dtype=np.float32) * 0.02,
        "bo": np.zeros(E, np.float32),
        "RWq": rng.standard_normal((G, E, E), dtype=np.float32) * 0.02,
        "Rbq": np.zeros((G, E), np.float32),
        "RWk": rng.standard_normal((G, E, E), dtype=np.float32) * 0.02,
        "Rbk": np.zeros((G, E), np.float32),
        "RWv": rng.standard_normal((G, E, E), dtype=np.float32) * 0.02,
        "Rbv": np.zeros((G, E), np.float32),
    }
    out = kernel(**ins)
    print("out", out.shape, out.dtype, float(np.abs(out).max()))
